# revision 7
# baseline (speedup 1.0000x reference)
"""Causal MoE (top-1) Trainium2 kernel.

Strategy (8 cores):
  Phase 1 (device): causal routing, split each batch across 2 cores
    (1024 tokens per core). logits[t] = (1/(t+1)) * cumsum(x @ rw)[t],
    computed chunk-pipelined: per 128-token chunk, r = x_chunk @ rw via
    PE, in-chunk prefix via a constant triangular matmul, cross-chunk
    prefix via an accumulated chunk-sum vector. Second-half cores get
    the first half's column-sum of x from the host (router is linear,
    so (sum x) @ rw == sum (x @ rw)). Softmax max-prob gate + argmax.
  Host: group token ids by expert, pack into 8 cores x 2 segments
    (sizes CA / CB = C - CA, chosen by a small search to minimize C);
    each segment has its own expert weight set, so a core can serve
    two experts (or one expert twice) -- near-perfect load balance
    even with skewed routing.
  Phase 2 (device): per-core dense FFN over C tokens in two segments:
    y = gate * (gelu_tanh(x @ w1 + b1) @ w2 + b2), bf16 weights/
    activations, fp32 PSUM accumulation, full hidden state resident
    in SBUF, single pass over the weights.
  Host: scatter rows back to [B, S, D].
"""

import numpy as np
from contextlib import ExitStack

import ml_dtypes

import concourse.bass as bass
import concourse.mybir as mybir
import concourse.tile as tile
from concourse.bass_utils import run_bass_kernel_spmd
from concourse.vector_clock import ScopedClock


def _run_spmd(nc, in_maps, core_ids, retries=2):
    """run_bass_kernel_spmd with retry on transient device faults."""
    last = None
    for attempt in range(retries + 1):
        try:
            return run_bass_kernel_spmd(nc, in_maps, core_ids=core_ids)
        except Exception as e:  # e.g. NRT_EXEC_UNIT_UNRECOVERABLE blips
            last = e
            if attempt == retries:
                raise
    raise last


B, S, D, F, E = 4, 2048, 1024, 4096, 4
NCORES = 8
ND = D // 128  # 8
NF = F // 128  # 32
SH = S // 2  # 1024 tokens per phase-1 core
NCH = SH // 128  # 8 chunks per phase-1 core
FP = mybir.dt.float32
BF = mybir.dt.bfloat16
AFT = mybir.ActivationFunctionType
AX = mybir.AxisListType

# ---------------------------------------------------------------------------
# Workaround: the walrus build here allows only 1 sync-wait per instruction
# (setupSyncWait "Too many sync wait commands"), while Tile may attach
# several. Split the extra waits onto carrier NoOps on the same engine,
# executed immediately before the instruction (same stream => same
# semantics). Applied both to the scheduled instruction lists and to the
# TileContext tail drain.
_PATCHED = False

_orig_lower_ordered_insts = tile.TileContext._lower_ordered_insts


def _split_waits_lower(self, ordered):
    nc = self.nc
    for insts in ordered.values():
        new = []
        for inst in insts:
            si = getattr(inst, "sync_info", None)
            eng = getattr(inst, "engine", None)
            if (
                si is not None
                and si.on_wait
                and len(si.on_wait) > 1
                and eng is not None
                and eng != mybir.EngineType.Unassigned
            ):
                waits = list(si.on_wait)
                si.on_wait.clear()
                si.on_wait.append(waits[-1])
                for w in waits[:-1]:
                    nop = mybir.InstNoOp(
                        name=nc.get_next_instruction_name(), ins=[], outs=[]
                    )
                    nop.engine = eng
                    nop.sync_info = mybir.SyncInfo(on_wait=[w], on_update=[])
                    new.append(nop)
            new.append(inst)
        insts[:] = new
    return _orig_lower_ordered_insts(self, ordered)


def _patched_drain_and_barrier(self, tick_clock, wait_clock):
    nc = self.nc
    drain_inst = nc.sync.drain()
    wait_clock.add_sem_waits(
        drain_inst.ins, ScopedClock({None: tick_clock.global_clock})
    )
    si = drain_inst.ins.sync_info
    if si is not None and len(si.on_wait) > 1:
        waits = list(si.on_wait)
        si.on_wait.clear()
        si.on_wait.append(waits[0])
        for w in waits[1:]:
            nop = nc.sync.nop(nofuse=True, hint="drain_wait_spill")
            nsi = nop.ins.sync_info
            if nsi is None:
                nop.ins.sync_info = mybir.SyncInfo(on_wait=[], on_update=[])
                nsi = nop.ins.sync_info
            nsi.on_wait.append(w)
    nc.all_engine_barrier()
    assert self.sems is not None
    popped = nc._tile_sem_poison_stack.pop()
    assert popped is self._sem_poison
    nc.clear_and_free_semaphores(list(self.sems.allocated().values()))
    nc.all_engine_barrier()


def _patch_tile_drain():
    global _PATCHED
    if not _PATCHED:
        tile.TileContext._drain_and_barrier = _patched_drain_and_barrier
        tile.TileContext._lower_ordered_insts = _split_waits_lower
        _PATCHED = True


# ---------------------------------------------------------------------------
# Phase 1: routing. Half a batch (1024 tokens) per core, chunk-pipelined.


def build_phase1(hw_loop=0):
    _patch_tile_drain()
    nc = bass.Bass("TRN2", num_devices=NCORES, debug=False)
    xp = nc.dram_tensor("xp", [NCH, 128, D], FP, kind="ExternalInput")
    rwp = nc.dram_tensor("rwp", [128, ND * E], FP, kind="ExternalInput")
    xhsp = nc.dram_tensor("xhsp", [128, ND], FP, kind="ExternalInput")
    invc = nc.dram_tensor("invc", [128, NCH], FP, kind="ExternalInput")
    trim = nc.dram_tensor("trim", [128, 128], FP, kind="ExternalInput")
    iotar = nc.dram_tensor("iotar", [128, E], FP, kind="ExternalInput")
    onec = nc.dram_tensor("onec", [128, 1], FP, kind="ExternalInput")
    oner = nc.dram_tensor("oner", [1, 128], FP, kind="ExternalInput")
    idx_out = nc.dram_tensor("idx_out", [128, NCH], FP, kind="ExternalOutput")
    gate_out = nc.dram_tensor("gate_out", [128, NCH], FP, kind="ExternalOutput")

    with tile.TileContext(nc) as tc, ExitStack() as ctx:
        cpool = ctx.enter_context(tc.tile_pool(name="const", bufs=1))
        xpool = ctx.enter_context(tc.tile_pool(name="x", bufs=3))
        rpool = ctx.enter_context(tc.tile_pool(name="r", bufs=3))
        apool = ctx.enter_context(tc.tile_pool(name="acc", bufs=3))
        wpool = ctx.enter_context(tc.tile_pool(name="work", bufs=3))
        ppool = ctx.enter_context(tc.tile_pool(name="persist", bufs=1))
        pspool = ctx.enter_context(tc.tile_pool(name="psum", bufs=3, space="PSUM"))

        loop_ctx = tc.For_i(0, hw_loop, 1) if hw_loop else None
        if loop_ctx is not None:
            loop_ctx.__enter__()

        rw_t = cpool.tile([128, ND * E], FP, name="rw", tag="rw")
        nc.scalar.dma_start(rw_t[:], rwp.ap()[:])
        xhs_t = cpool.tile([128, ND], FP, name="xhs", tag="xhs")
        nc.scalar.dma_start(xhs_t[:], xhsp.ap()[:])
        invc_t = cpool.tile([128, NCH], FP, name="invc", tag="invc")
        nc.scalar.dma_start(invc_t[:], invc.ap()[:])
        tri_t = cpool.tile([128, 128], FP, name="tri", tag="tri")
        nc.scalar.dma_start(tri_t[:], trim.ap()[:])
        iota_t = cpool.tile([128, E], FP, name="iota", tag="iota")
        nc.scalar.dma_start(iota_t[:], iotar.ap()[:])
        onec_t = cpool.tile([128, 1], FP, name="onec", tag="onec")
        nc.scalar.dma_start(onec_t[:], onec.ap()[:])
        oner_t = cpool.tile([1, 128], FP, name="oner", tag="oner")
        nc.scalar.dma_start(oner_t[:], oner.ap()[:])

        idx_t = ppool.tile([128, NCH], FP, name="idx", tag="idx")
        gate_t = ppool.tile([128, NCH], FP, name="gate", tag="gate")

        # prefix chunk-sum accumulator pc[0] = xhalfsum @ rw (zeros for
        # first-half cores)
        si_ps = pspool.tile([1, E], FP, name="si_ps", tag="s", bufs=2)
        for d in range(ND):
            nc.tensor.matmul(
                si_ps[:],
                xhs_t[:, d : d + 1],
                rw_t[:, d * E : (d + 1) * E],
                start=(d == 0),
                stop=(d == ND - 1),
            )
        pc = apool.tile([1, E], FP, name="pc", tag="pc")
        nc.vector.tensor_copy(pc[:], si_ps[:])

        prev = None  # (r_sb, pc_for_chunk, c)

        def stage2(r_sb, pc_cur, c):
            # chunk sum -> next prefix accumulator
            s_ps = pspool.tile([1, E], FP, name="s_ps", tag="s", bufs=2)
            nc.tensor.matmul(s_ps[:], onec_t[:], r_sb[:], start=True, stop=True)
            pc_next = apool.tile([1, E], FP, name="pc", tag="pc")
            nc.vector.tensor_add(pc_next[:], pc_cur[:], s_ps[:])
            # logits: in-chunk prefix + broadcast prefix accumulator
            lg_ps = pspool.tile([128, E], FP, name="lg_ps", tag="lg")
            nc.tensor.matmul(lg_ps[:], tri_t[:], r_sb[:], start=True, stop=False)
            nc.tensor.matmul(lg_ps[:], oner_t[:], pc_cur[:], start=False, stop=True)
            lg_sb = wpool.tile([128, E], FP, name="lg_sb", tag="lg_sb")
            nc.vector.tensor_scalar_mul(lg_sb[:], lg_ps[:], invc_t[:, c : c + 1])
            # softmax: gate = 1/sum(exp(lg - max)); idx via first-max trick
            m_t = wpool.tile([128, 1], FP, name="m", tag="m")
            nc.vector.reduce_max(m_t[:], lg_sb[:], axis=AX.X)
            negm = wpool.tile([128, 1], FP, name="negm", tag="negm")
            nc.vector.tensor_scalar_mul(negm[:], m_t[:], -1.0)
            et = wpool.tile([128, E], FP, name="et", tag="et")
            nc.scalar.activation(et[:], lg_sb[:], AFT.Exp, bias=negm[:], scale=1.0)
            ssum = wpool.tile([128, 1], FP, name="ssum", tag="ssum")
            nc.vector.reduce_sum(ssum[:], et[:], axis=AX.X)
            nc.vector.reciprocal(gate_t[:, c : c + 1], ssum[:])
            msk = wpool.tile([128, E], FP, name="msk", tag="msk")
            nc.vector.tensor_scalar(
                msk[:], lg_sb[:], m_t[:], None, op0=mybir.AluOpType.is_equal
            )
            nc.vector.tensor_mul(msk[:], msk[:], iota_t[:])
            rev = wpool.tile([128, 1], FP, name="rev", tag="rev")
            nc.vector.reduce_max(rev[:], msk[:], axis=AX.X)
            nc.scalar.activation(
                idx_t[:, c : c + 1], rev[:], AFT.Copy, bias=3.0, scale=-1.0
            )
            return pc_next

        for c in range(NCH):
            xc = xpool.tile([128, D], FP, name="xc", tag="xc")
            (nc.sync if c % 2 == 0 else nc.scalar).dma_start(xc[:], xp.ap()[c])
            r_ps = pspool.tile([128, E], FP, name="r_ps", tag="r")
            for d in range(ND):
                nc.tensor.matmul(
                    r_ps[:],
                    xc[:, d * 128 : (d + 1) * 128],
                    rw_t[:, d * E : (d + 1) * E],
                    start=(d == 0),
                    stop=(d == ND - 1),
                )
            r_sb = rpool.tile([128, E], FP, name="r_sb", tag="r_sb")
            nc.vector.tensor_copy(r_sb[:], r_ps[:])
            if prev is not None:
                pc = stage2(*prev)
            prev = (r_sb, pc, c)
        pc = stage2(*prev)

        nc.sync.dma_start(idx_out.ap()[:], idx_t[:])
        nc.sync.dma_start(gate_out.ap()[:], gate_t[:])
        if loop_ctx is not None:
            loop_ctx.__exit__(None, None, None)
    return nc


def phase1_constants():
    q = np.arange(128)
    tri = (q[:, None] <= q[None, :]).astype(np.float32)
    iotar = np.broadcast_to(
        (3.0 - np.arange(E, dtype=np.float32))[None, :], (128, E)
    ).copy()
    onec = np.ones((128, 1), np.float32)
    oner = np.ones((1, 128), np.float32)
    return {
        "trim": np.ascontiguousarray(tri),
        "iotar": iotar,
        "onec": onec,
        "oner": oner,
    }


def phase1_in_maps(x, router_w):
    """Per-core inputs: core = half*B + b handles x[b, half*SH:(half+1)*SH]."""
    consts = phase1_constants()
    rwp = np.ascontiguousarray(
        router_w.reshape(ND, 128, E).transpose(1, 0, 2).reshape(128, ND * E)
    )
    in_maps = []
    for core in range(NCORES):
        b, half = core % B, core // B
        off = half * SH
        xh = x[b, off : off + SH]  # [SH, D]
        xp = np.ascontiguousarray(
            xh.reshape(NCH, 128, ND, 128).transpose(0, 3, 2, 1).reshape(NCH, 128, D)
        )
        if half:
            xhs = x[b, :SH].sum(0, dtype=np.float64).astype(np.float32)
        else:
            xhs = np.zeros((D,), np.float32)
        xhsp = np.ascontiguousarray(xhs.reshape(ND, 128).T)
        t = off + np.arange(SH)
        invc = np.ascontiguousarray(
            (1.0 / (t + 1.0)).astype(np.float32).reshape(NCH, 128).T
        )
        in_maps.append(
            {"xp": xp, "rwp": rwp, "xhsp": xhsp, "invc": invc, **consts}
        )
    return in_maps


def run_phase1(x, router_w):
    nc = build_phase1()
    res = _run_spmd(nc, phase1_in_maps(x, router_w), list(range(NCORES)))
    idx = np.empty((B, S), np.int32)
    gate = np.empty((B, S), np.float32)
    for core in range(NCORES):
        b, half = core % B, core // B
        off = half * SH
        r = res.results[core]
        idx[b, off : off + SH] = (
            np.rint(r["idx_out"]).astype(np.int32).T.reshape(SH)
        )
        gate[b, off : off + SH] = r["gate_out"].T.reshape(SH).astype(np.float32)
    return idx, gate


# ---------------------------------------------------------------------------
# Phase 2: per-core FFN over C tokens in two segments (A: [0, CA) and
# B: [CA, C)), each with its own expert weight set.


def plan_two_segments(counts):
    """Find C = CA + CB (multiples of 8) minimizing effective PE cost such
    that each expert's token count fits in a_e A-slots (capacity CA) +
    b_e B-slots (capacity CB) with sum(a) <= NCORES, sum(b) <= NCORES.
    A segment shorter than ~257 tokens can't hide the 128-col LDWEIGHTS
    under its matmuls, so short CB is penalized. Returns (C, CA, alloc)
    where alloc[e] = (a_e, b_e)."""
    counts = [int(c) for c in counts]

    def feasible(CA, CB):
        # DP over experts: reachable (A_used, B_used)
        reach = {(0, 0): []}
        for cnt in counts:
            nxt = {}
            for (au, bu), al in reach.items():
                for a in range(0, NCORES - au + 1):
                    rem = cnt - a * CA
                    if rem <= 0:
                        bneed = 0
                    elif CB > 0:
                        bneed = -(-rem // CB)
                    else:
                        continue
                    if bu + bneed > NCORES:
                        continue
                    key = (au + a, bu + bneed)
                    if key not in nxt:
                        nxt[key] = al + [(a, bneed)]
            reach = nxt
            if not reach:
                return None
        return next(iter(reach.values()))

    total = sum(counts)
    cmin = max(128, (-(-total // NCORES) + 7) // 8 * 8)
    best = None  # (c_eff, C, CA, alloc)
    for C in range(cmin, 4097, 8):
        if best is not None and C >= best[0]:
            break
        for CA in range(C // 2, C + 1, 8):
            CB = C - CA
            al = feasible(CA, CB)
            if al is None:
                continue
            pen = max(0, 257 - CB) if CB > 0 else 0
            c_eff = C + pen
            if best is None or c_eff < best[0]:
                best = (c_eff, C, CA, al)
            break  # larger CA at same C is never better
    if best is None:
        raise RuntimeError("no feasible two-segment plan")
    return best[1], best[2], best[3]


def pack_w1(w1e):
    """[D, F] -> [NF, 128, D]: out[fi, k, d*128+m] = w1e[d*128+k, fi*128+m]"""
    return np.ascontiguousarray(
        w1e.reshape(ND, 128, NF, 128).transpose(2, 1, 0, 3).reshape(NF, 128, D)
    )


def pack_w2(w2e):
    """[F, D] -> [ND, 128, F]: out[dd, k, fi*128+m] = w2e[fi*128+k, dd*128+m]"""
    return np.ascontiguousarray(
        w2e.reshape(NF, 128, ND, 128).transpose(2, 1, 0, 3).reshape(ND, 128, F)
    )


def _chunks(o0, n, cap=512):
    out = []
    o = 0
    while o < n:
        m = min(cap, n - o)
        out.append((o0 + o, m))
        o += m
    return out


def build_phase2(C, CA, hw_loop=0):
    _patch_tile_drain()
    nc = bass.Bass("TRN2", num_devices=NCORES, debug=False)
    CB = C - CA
    xgT = nc.dram_tensor("xgT", [ND, 128, C], BF, kind="ExternalInput")
    w1A = nc.dram_tensor("w1A", [NF, 128, D], BF, kind="ExternalInput")
    w1B = nc.dram_tensor("w1B", [NF, 128, D], BF, kind="ExternalInput")
    w2A = nc.dram_tensor("w2A", [ND, 128, F], BF, kind="ExternalInput")
    w2B = nc.dram_tensor("w2B", [ND, 128, F], BF, kind="ExternalInput")
    b1r = nc.dram_tensor("b1r", [128, 2 * NF], FP, kind="ExternalInput")
    b2r = nc.dram_tensor("b2r", [128, 2 * ND], FP, kind="ExternalInput")
    gateb = nc.dram_tensor("gateb", [128, C], FP, kind="ExternalInput")
    ygT = nc.dram_tensor("ygT", [ND, 128, C], FP, kind="ExternalOutput")

    chA = _chunks(0, CA)
    chB = _chunks(CA, CB)
    segs = [(0, chA), (1, chB)] if CB > 0 else [(0, chA)]

    with tile.TileContext(nc) as tc, ExitStack() as ctx:
        cpool = ctx.enter_context(tc.tile_pool(name="const", bufs=1))
        hpool = ctx.enter_context(tc.tile_pool(name="h", bufs=1))
        w1pool = ctx.enter_context(tc.tile_pool(name="w1", bufs=3))
        w2pool = ctx.enter_context(tc.tile_pool(name="w2", bufs=2))
        ypool = ctx.enter_context(tc.tile_pool(name="y", bufs=6))
        pspool = ctx.enter_context(tc.tile_pool(name="psum", bufs=8, space="PSUM"))

        loop_ctx = tc.For_i(0, hw_loop, 1) if hw_loop else None
        if loop_ctx is not None:
            loop_ctx.__enter__()

        xg_t = []
        for d in range(ND):
            t = cpool.tile([128, C], BF, name=f"xg{d}", tag=f"xg{d}")
            nc.scalar.dma_start(t[:], xgT.ap()[d])
            xg_t.append(t)
        gate_t = cpool.tile([128, C], FP, name="gate", tag="gate")
        nc.scalar.dma_start(gate_t[:], gateb.ap()[:])
        b1_t = cpool.tile([128, 2 * NF], FP, name="b1", tag="b1")
        nc.scalar.dma_start(b1_t[:], b1r.ap()[:])
        b2_t = cpool.tile([128, 2 * ND], FP, name="b2", tag="b2")
        nc.scalar.dma_start(b2_t[:], b2r.ap()[:])

        h_t = [
            hpool.tile([128, C], BF, name=f"h{fi}", tag=f"h{fi}")
            for fi in range(NF)
        ]

        # L1: h = gelu(x @ w1 + b1), one f-tile (128 cols of F) at a time
        w1_dram = [w1A, w1B]
        for fi in range(NF):
            wts = {}
            for s, ch in segs:
                wt = w1pool.tile([128, D], BF, name=f"w1s{s}", tag=f"w1s{s}")
                nc.sync.dma_start(wt[:], w1_dram[s].ap()[fi])
                wts[s] = wt
            pss = {
                (s, j): pspool.tile([128, n], FP, name="ps", tag="ps")
                for s, ch in segs
                for j, (o, n) in enumerate(ch)
            }
            for d in range(ND):
                for s, ch in segs:
                    for j, (o, n) in enumerate(ch):
                        nc.tensor.matmul(
                            pss[s, j][:],
                            wts[s][:, d * 128 : (d + 1) * 128],
                            xg_t[d][:, o : o + n],
                            start=(d == 0),
                            stop=(d == ND - 1),
                        )
            for s, ch in segs:
                for j, (o, n) in enumerate(ch):
                    nc.scalar.activation(
                        h_t[fi][:, o : o + n],
                        pss[s, j][:],
                        AFT.Gelu_apprx_tanh,
                        bias=b1_t[:, s * NF + fi : s * NF + fi + 1],
                        scale=1.0,
                    )

        # L2: y = gate * (h @ w2 + b2), one d-tile (128 cols of D) at a time
        w2_dram = [w2A, w2B]
        for dd in range(ND):
            wts = {}
            for s, ch in segs:
                wt = w2pool.tile([128, F], BF, name=f"w2s{s}", tag=f"w2s{s}")
                nc.sync.dma_start(wt[:], w2_dram[s].ap()[dd])
                wts[s] = wt
            qss = {
                (s, j): pspool.tile([128, n], FP, name="qs", tag="ps")
                for s, ch in segs
                for j, (o, n) in enumerate(ch)
            }
            for fi in range(NF):
                for s, ch in segs:
                    for j, (o, n) in enumerate(ch):
                        nc.tensor.matmul(
                            qss[s, j][:],
                            wts[s][:, fi * 128 : (fi + 1) * 128],
                            h_t[fi][:, o : o + n],
                            start=(fi == 0),
                            stop=(fi == NF - 1),
                        )
            for s, ch in segs:
                for j, (o, n) in enumerate(ch):
                    yt = ypool.tile([128, n], FP, name="yt", tag="yt")
                    nc.vector.tensor_scalar_add(
                        yt[:], qss[s, j][:], b2_t[:, s * ND + dd : s * ND + dd + 1]
                    )
                    nc.vector.tensor_mul(yt[:], yt[:], gate_t[:, o : o + n])
                    nc.scalar.dma_start(ygT.ap()[dd][:, o : o + n], yt[:])

        if loop_ctx is not None:
            loop_ctx.__exit__(None, None, None)
    return nc


def phase2_assign(flat_idx):
    """Pack tokens into 8 cores x 2 segments. Returns (C, CA, slots) where
    slots[core] = (expert_A, ids_A, expert_B, ids_B)."""
    counts = np.bincount(flat_idx, minlength=E)
    C, CA, alloc = plan_two_segments(counts)
    CB = C - CA
    a_slots, b_slots = [], []
    for e in range(E):
        ids_e = np.nonzero(flat_idx == e)[0]
        a_e, b_e = alloc[e]
        pos = 0
        for _ in range(a_e):
            take = min(CA, len(ids_e) - pos)
            a_slots.append((e, ids_e[pos : pos + take]))
            pos += take
        for _ in range(b_e):
            take = min(CB, len(ids_e) - pos)
            b_slots.append((e, ids_e[pos : pos + take]))
            pos += take
        assert pos == len(ids_e)
    empty = np.zeros((0,), np.int64)
    while len(a_slots) < NCORES:
        a_slots.append((0, empty))
    while len(b_slots) < NCORES:
        b_slots.append((0, empty))
    slots = [
        (a_slots[k][0], a_slots[k][1], b_slots[k][0], b_slots[k][1])
        for k in range(NCORES)
    ]
    return C, CA, slots


def phase2_in_maps(x, w1, b1, w2, b2, gate, C, CA, slots):
    flat_x = x.reshape(B * S, D)
    flat_gate = gate.reshape(-1).astype(np.float32)
    used = sorted({e for ea, _, eb, _ in slots for e in (ea, eb)})
    w1p = {e: pack_w1(w1[e].astype(ml_dtypes.bfloat16)) for e in used}
    w2p = {e: pack_w2(w2[e].astype(ml_dtypes.bfloat16)) for e in used}
    b1p = {e: np.ascontiguousarray(b1[e].reshape(NF, 128).T) for e in used}
    b2p = {e: np.ascontiguousarray(b2[e].reshape(ND, 128).T) for e in used}
    in_maps = []
    for eA, idsA, eB, idsB in slots:
        xg = np.zeros((C, D), np.float32)
        gt = np.zeros((C,), np.float32)
        if len(idsA):
            xg[: len(idsA)] = flat_x[idsA]
            gt[: len(idsA)] = flat_gate[idsA]
        if len(idsB):
            xg[CA : CA + len(idsB)] = flat_x[idsB]
            gt[CA : CA + len(idsB)] = flat_gate[idsB]
        xgT = np.ascontiguousarray(
            xg.T.astype(ml_dtypes.bfloat16).reshape(ND, 128, C)
        )
        in_maps.append(
            {
                "xgT": xgT,
                "w1A": w1p[eA],
                "w1B": w1p[eB],
                "w2A": w2p[eA],
                "w2B": w2p[eB],
                "b1r": np.ascontiguousarray(np.concatenate([b1p[eA], b1p[eB]], 1)),
                "b2r": np.ascontiguousarray(np.concatenate([b2p[eA], b2p[eB]], 1)),
                "gateb": np.broadcast_to(gt[None, :], (128, C)).copy(),
            }
        )
    return in_maps


def kernel(x, router_w, w1, b1, w2, b2):
    x = np.asarray(x, np.float32)
    router_w = np.asarray(router_w, np.float32)
    w1 = np.asarray(w1, np.float32)
    b1 = np.asarray(b1, np.float32)
    w2 = np.asarray(w2, np.float32)
    b2 = np.asarray(b2, np.float32)

    idx, gate = run_phase1(x, router_w)  # [B, S] each

    flat_idx = idx.reshape(-1)
    C, CA, slots = phase2_assign(flat_idx)
    nc2 = build_phase2(C, CA)
    in_maps = phase2_in_maps(x, w1, b1, w2, b2, gate, C, CA, slots)
    res2 = _run_spmd(nc2, in_maps, list(range(NCORES)))

    y_flat = np.zeros((B * S, D), np.float32)
    for core, (eA, idsA, eB, idsB) in enumerate(slots):
        ygT = res2.results[core]["ygT"].reshape(D, C)
        if len(idsA):
            y_flat[idsA] = ygT[:, : len(idsA)].T
        if len(idsB):
            y_flat[idsB] = ygT[:, CA : CA + len(idsB)].T
    return y_flat.reshape(B, S, D)


# revision 11
# speedup vs baseline: 1.2019x; 1.2019x over previous
"""Causal MoE (top-1) Trainium2 kernel.

Strategy (8 cores):
  Phase 1 (device): causal routing, split each batch across 2 cores
    (1024 tokens per core). logits[t] = (1/(t+1)) * cumsum(x @ rw)[t],
    computed chunk-pipelined: per 128-token chunk, r = x_chunk @ rw via
    PE, in-chunk prefix via a constant triangular matmul, cross-chunk
    prefix via an accumulated chunk-sum vector. Second-half cores get
    the first half's column-sum of x from the host (router is linear,
    so (sum x) @ rw == sum (x @ rw)). Softmax max-prob gate + argmax.
  Host: group token ids by expert, pack into 8 cores x 2 segments
    (sizes CA / CB = C - CA, chosen by a small search to minimize C);
    each segment has its own expert weight set, so a core can serve
    two experts (or one expert twice) -- near-perfect load balance
    even with skewed routing.
  Phase 2 (device): per-core dense FFN over C tokens in two segments:
    y = gate * (gelu_tanh(x @ w1 + b1) @ w2 + b2), bf16 weights/
    activations, fp32 PSUM accumulation, full hidden state resident
    in SBUF, single pass over the weights.
  Host: scatter rows back to [B, S, D].
"""

import numpy as np
from contextlib import ExitStack

import ml_dtypes

import concourse.bass as bass
import concourse.mybir as mybir
import concourse.tile as tile
from concourse.bass_utils import run_bass_kernel_spmd
from concourse.vector_clock import ScopedClock


def _run_spmd(nc, in_maps, core_ids, retries=2):
    """run_bass_kernel_spmd with retry on transient device faults."""
    last = None
    for attempt in range(retries + 1):
        try:
            return run_bass_kernel_spmd(nc, in_maps, core_ids=core_ids)
        except Exception as e:  # e.g. NRT_EXEC_UNIT_UNRECOVERABLE blips
            last = e
            if attempt == retries:
                raise
    raise last


B, S, D, F, E = 4, 2048, 1024, 4096, 4
NCORES = 8
ND = D // 128  # 8
NF = F // 128  # 32
SH = S // 2  # 1024 tokens per phase-1 core
NCH = SH // 128  # 8 chunks per phase-1 core
FP = mybir.dt.float32
BF = mybir.dt.bfloat16
AFT = mybir.ActivationFunctionType
AX = mybir.AxisListType

# ---------------------------------------------------------------------------
# Workaround: the walrus build here allows only 1 sync-wait per instruction
# (setupSyncWait "Too many sync wait commands"), while Tile may attach
# several. Split the extra waits onto carrier NoOps on the same engine,
# executed immediately before the instruction (same stream => same
# semantics). Applied both to the scheduled instruction lists and to the
# TileContext tail drain.
_PATCHED = False

_orig_lower_ordered_insts = tile.TileContext._lower_ordered_insts


def _split_waits_lower(self, ordered):
    nc = self.nc
    for insts in ordered.values():
        new = []
        for inst in insts:
            si = getattr(inst, "sync_info", None)
            eng = getattr(inst, "engine", None)
            if (
                si is not None
                and si.on_wait
                and len(si.on_wait) > 1
                and eng is not None
                and eng != mybir.EngineType.Unassigned
            ):
                waits = list(si.on_wait)
                si.on_wait.clear()
                si.on_wait.append(waits[-1])
                for w in waits[:-1]:
                    nop = mybir.InstNoOp(
                        name=nc.get_next_instruction_name(), ins=[], outs=[]
                    )
                    nop.engine = eng
                    nop.sync_info = mybir.SyncInfo(on_wait=[w], on_update=[])
                    new.append(nop)
            new.append(inst)
        insts[:] = new
    return _orig_lower_ordered_insts(self, ordered)


def _patched_drain_and_barrier(self, tick_clock, wait_clock):
    nc = self.nc
    drain_inst = nc.sync.drain()
    wait_clock.add_sem_waits(
        drain_inst.ins, ScopedClock({None: tick_clock.global_clock})
    )
    si = drain_inst.ins.sync_info
    if si is not None and len(si.on_wait) > 1:
        waits = list(si.on_wait)
        si.on_wait.clear()
        si.on_wait.append(waits[0])
        for w in waits[1:]:
            nop = nc.sync.nop(nofuse=True, hint="drain_wait_spill")
            nsi = nop.ins.sync_info
            if nsi is None:
                nop.ins.sync_info = mybir.SyncInfo(on_wait=[], on_update=[])
                nsi = nop.ins.sync_info
            nsi.on_wait.append(w)
    nc.all_engine_barrier()
    assert self.sems is not None
    popped = nc._tile_sem_poison_stack.pop()
    assert popped is self._sem_poison
    nc.clear_and_free_semaphores(list(self.sems.allocated().values()))
    nc.all_engine_barrier()


def _patch_tile_drain():
    global _PATCHED
    if not _PATCHED:
        tile.TileContext._drain_and_barrier = _patched_drain_and_barrier
        tile.TileContext._lower_ordered_insts = _split_waits_lower
        _PATCHED = True


# ---------------------------------------------------------------------------
# Phase 1: routing. Half a batch (1024 tokens) per core, chunk-pipelined.


def build_phase1(hw_loop=0):
    _patch_tile_drain()
    nc = bass.Bass("TRN2", num_devices=NCORES, debug=False)
    xp = nc.dram_tensor("xp", [NCH, 128, D], FP, kind="ExternalInput")
    rwp = nc.dram_tensor("rwp", [128, ND * E], FP, kind="ExternalInput")
    xhsp = nc.dram_tensor("xhsp", [128, ND], FP, kind="ExternalInput")
    invc = nc.dram_tensor("invc", [128, NCH], FP, kind="ExternalInput")
    trim = nc.dram_tensor("trim", [128, 128], FP, kind="ExternalInput")
    iotar = nc.dram_tensor("iotar", [128, E], FP, kind="ExternalInput")
    onec = nc.dram_tensor("onec", [128, 1], FP, kind="ExternalInput")
    oner = nc.dram_tensor("oner", [1, 128], FP, kind="ExternalInput")
    idx_out = nc.dram_tensor("idx_out", [128, NCH], FP, kind="ExternalOutput")
    gate_out = nc.dram_tensor("gate_out", [128, NCH], FP, kind="ExternalOutput")

    with tile.TileContext(nc) as tc, ExitStack() as ctx:
        cpool = ctx.enter_context(tc.tile_pool(name="const", bufs=1))
        xpool = ctx.enter_context(tc.tile_pool(name="x", bufs=3))
        rpool = ctx.enter_context(tc.tile_pool(name="r", bufs=3))
        apool = ctx.enter_context(tc.tile_pool(name="acc", bufs=3))
        wpool = ctx.enter_context(tc.tile_pool(name="work", bufs=3))
        ppool = ctx.enter_context(tc.tile_pool(name="persist", bufs=1))
        pspool = ctx.enter_context(tc.tile_pool(name="psum", bufs=3, space="PSUM"))

        loop_ctx = tc.For_i(0, hw_loop, 1) if hw_loop else None
        if loop_ctx is not None:
            loop_ctx.__enter__()

        rw_t = cpool.tile([128, ND * E], FP, name="rw", tag="rw")
        nc.scalar.dma_start(rw_t[:], rwp.ap()[:])
        xhs_t = cpool.tile([128, ND], FP, name="xhs", tag="xhs")
        nc.scalar.dma_start(xhs_t[:], xhsp.ap()[:])
        invc_t = cpool.tile([128, NCH], FP, name="invc", tag="invc")
        nc.scalar.dma_start(invc_t[:], invc.ap()[:])
        tri_t = cpool.tile([128, 128], FP, name="tri", tag="tri")
        nc.scalar.dma_start(tri_t[:], trim.ap()[:])
        iota_t = cpool.tile([128, E], FP, name="iota", tag="iota")
        nc.scalar.dma_start(iota_t[:], iotar.ap()[:])
        onec_t = cpool.tile([128, 1], FP, name="onec", tag="onec")
        nc.scalar.dma_start(onec_t[:], onec.ap()[:])
        oner_t = cpool.tile([1, 128], FP, name="oner", tag="oner")
        nc.scalar.dma_start(oner_t[:], oner.ap()[:])

        idx_t = ppool.tile([128, NCH], FP, name="idx", tag="idx")
        gate_t = ppool.tile([128, NCH], FP, name="gate", tag="gate")

        # prefix chunk-sum accumulator pc[0] = xhalfsum @ rw (zeros for
        # first-half cores)
        si_ps = pspool.tile([1, E], FP, name="si_ps", tag="s", bufs=2)
        for d in range(ND):
            nc.tensor.matmul(
                si_ps[:],
                xhs_t[:, d : d + 1],
                rw_t[:, d * E : (d + 1) * E],
                start=(d == 0),
                stop=(d == ND - 1),
            )
        pc = apool.tile([1, E], FP, name="pc", tag="pc")
        nc.vector.tensor_copy(pc[:], si_ps[:])

        prev = None  # (r_sb, pc_for_chunk, c)

        def stage2(r_sb, pc_cur, c):
            # chunk sum -> next prefix accumulator
            s_ps = pspool.tile([1, E], FP, name="s_ps", tag="s", bufs=2)
            nc.tensor.matmul(s_ps[:], onec_t[:], r_sb[:], start=True, stop=True)
            pc_next = apool.tile([1, E], FP, name="pc", tag="pc")
            nc.vector.tensor_add(pc_next[:], pc_cur[:], s_ps[:])
            # logits: in-chunk prefix + broadcast prefix accumulator
            lg_ps = pspool.tile([128, E], FP, name="lg_ps", tag="lg")
            nc.tensor.matmul(lg_ps[:], tri_t[:], r_sb[:], start=True, stop=False)
            nc.tensor.matmul(lg_ps[:], oner_t[:], pc_cur[:], start=False, stop=True)
            lg_sb = wpool.tile([128, E], FP, name="lg_sb", tag="lg_sb")
            nc.vector.tensor_scalar_mul(lg_sb[:], lg_ps[:], invc_t[:, c : c + 1])
            # softmax: gate = 1/sum(exp(lg - max)); idx via first-max trick
            m_t = wpool.tile([128, 1], FP, name="m", tag="m")
            nc.vector.reduce_max(m_t[:], lg_sb[:], axis=AX.X)
            negm = wpool.tile([128, 1], FP, name="negm", tag="negm")
            nc.vector.tensor_scalar_mul(negm[:], m_t[:], -1.0)
            et = wpool.tile([128, E], FP, name="et", tag="et")
            nc.scalar.activation(et[:], lg_sb[:], AFT.Exp, bias=negm[:], scale=1.0)
            ssum = wpool.tile([128, 1], FP, name="ssum", tag="ssum")
            nc.vector.reduce_sum(ssum[:], et[:], axis=AX.X)
            nc.vector.reciprocal(gate_t[:, c : c + 1], ssum[:])
            msk = wpool.tile([128, E], FP, name="msk", tag="msk")
            nc.vector.tensor_scalar(
                msk[:], lg_sb[:], m_t[:], None, op0=mybir.AluOpType.is_equal
            )
            nc.vector.tensor_mul(msk[:], msk[:], iota_t[:])
            rev = wpool.tile([128, 1], FP, name="rev", tag="rev")
            nc.vector.reduce_max(rev[:], msk[:], axis=AX.X)
            nc.scalar.activation(
                idx_t[:, c : c + 1], rev[:], AFT.Copy, bias=3.0, scale=-1.0
            )
            return pc_next

        for c in range(NCH):
            xc = xpool.tile([128, D], FP, name="xc", tag="xc")
            nc.sync.dma_start(xc[:], xp.ap()[c])
            r_ps = pspool.tile([128, E], FP, name="r_ps", tag="r")
            for d in range(ND):
                nc.tensor.matmul(
                    r_ps[:],
                    xc[:, d * 128 : (d + 1) * 128],
                    rw_t[:, d * E : (d + 1) * E],
                    start=(d == 0),
                    stop=(d == ND - 1),
                )
            r_sb = rpool.tile([128, E], FP, name="r_sb", tag="r_sb")
            nc.vector.tensor_copy(r_sb[:], r_ps[:])
            if prev is not None:
                pc = stage2(*prev)
            prev = (r_sb, pc, c)
        pc = stage2(*prev)

        nc.sync.dma_start(idx_out.ap()[:], idx_t[:])
        nc.sync.dma_start(gate_out.ap()[:], gate_t[:])
        if loop_ctx is not None:
            loop_ctx.__exit__(None, None, None)
    return nc


def phase1_constants():
    q = np.arange(128)
    tri = (q[:, None] <= q[None, :]).astype(np.float32)
    iotar = np.broadcast_to(
        (3.0 - np.arange(E, dtype=np.float32))[None, :], (128, E)
    ).copy()
    onec = np.ones((128, 1), np.float32)
    oner = np.ones((1, 128), np.float32)
    return {
        "trim": np.ascontiguousarray(tri),
        "iotar": iotar,
        "onec": onec,
        "oner": oner,
    }


def phase1_in_maps(x, router_w):
    """Per-core inputs: core = half*B + b handles x[b, half*SH:(half+1)*SH]."""
    consts = phase1_constants()
    rwp = np.ascontiguousarray(
        router_w.reshape(ND, 128, E).transpose(1, 0, 2).reshape(128, ND * E)
    )
    in_maps = []
    for core in range(NCORES):
        b, half = core % B, core // B
        off = half * SH
        xh = x[b, off : off + SH]  # [SH, D]
        xp = np.ascontiguousarray(
            xh.reshape(NCH, 128, ND, 128).transpose(0, 3, 2, 1).reshape(NCH, 128, D)
        )
        if half:
            xhs = x[b, :SH].sum(0, dtype=np.float64).astype(np.float32)
        else:
            xhs = np.zeros((D,), np.float32)
        xhsp = np.ascontiguousarray(xhs.reshape(ND, 128).T)
        t = off + np.arange(SH)
        invc = np.ascontiguousarray(
            (1.0 / (t + 1.0)).astype(np.float32).reshape(NCH, 128).T
        )
        in_maps.append(
            {"xp": xp, "rwp": rwp, "xhsp": xhsp, "invc": invc, **consts}
        )
    return in_maps


def run_phase1(x, router_w):
    nc = build_phase1()
    res = _run_spmd(nc, phase1_in_maps(x, router_w), list(range(NCORES)))
    idx = np.empty((B, S), np.int32)
    gate = np.empty((B, S), np.float32)
    for core in range(NCORES):
        b, half = core % B, core // B
        off = half * SH
        r = res.results[core]
        idx[b, off : off + SH] = (
            np.rint(r["idx_out"]).astype(np.int32).T.reshape(SH)
        )
        gate[b, off : off + SH] = r["gate_out"].T.reshape(SH).astype(np.float32)
    return idx, gate


# ---------------------------------------------------------------------------
# Phase 2: per-core FFN over C tokens in two segments (A: [0, CA) and
# B: [CA, C)), each with its own expert weight set.


def plan_two_segments(counts):
    """Find C = CA + CB (multiples of 8) minimizing effective PE cost such
    that each expert's token count fits in a_e A-slots (capacity CA) +
    b_e B-slots (capacity CB) with sum(a) <= NCORES, sum(b) <= NCORES.
    A segment shorter than ~257 tokens can't hide the 128-col LDWEIGHTS
    under its matmuls, so short CB is penalized. Returns (C, CA, alloc)
    where alloc[e] = (a_e, b_e)."""
    counts = [int(c) for c in counts]

    def feasible(CA, CB):
        # DP over experts: reachable (A_used, B_used)
        reach = {(0, 0): []}
        for cnt in counts:
            nxt = {}
            for (au, bu), al in reach.items():
                for a in range(0, NCORES - au + 1):
                    rem = cnt - a * CA
                    if rem <= 0:
                        bneed = 0
                    elif CB > 0:
                        bneed = -(-rem // CB)
                    else:
                        continue
                    if bu + bneed > NCORES:
                        continue
                    key = (au + a, bu + bneed)
                    if key not in nxt:
                        nxt[key] = al + [(a, bneed)]
            reach = nxt
            if not reach:
                return None
        return next(iter(reach.values()))

    total = sum(counts)
    cmin = max(128, (-(-total // NCORES) + 7) // 8 * 8)
    best = None  # (c_eff, C, CA, alloc)
    for C in range(cmin, 4097, 8):
        if best is not None and C >= best[0]:
            break
        for CA in range(C // 2, C + 1, 8):
            CB = C - CA
            al = feasible(CA, CB)
            if al is None:
                continue
            pen = max(0, 257 - CB) if CB > 0 else 0
            c_eff = C + pen
            if best is None or c_eff < best[0]:
                best = (c_eff, C, CA, al)
            break  # larger CA at same C is never better
    if best is None:
        raise RuntimeError("no feasible two-segment plan")
    return best[1], best[2], best[3]


def pack_w1(w1e):
    """[D, F] -> [NF, 128, D]: out[fi, k, d*128+m] = w1e[d*128+k, fi*128+m]"""
    return np.ascontiguousarray(
        w1e.reshape(ND, 128, NF, 128).transpose(2, 1, 0, 3).reshape(NF, 128, D)
    )


def pack_w2(w2e):
    """[F, D] -> [ND, 128, F]: out[dd, k, fi*128+m] = w2e[fi*128+k, dd*128+m]"""
    return np.ascontiguousarray(
        w2e.reshape(NF, 128, ND, 128).transpose(2, 1, 0, 3).reshape(ND, 128, F)
    )


def _chunks(o0, n, cap=512):
    """Split [o0, o0+n) into equal chunks <= cap. Equal sizes keep every
    chunk >= 257 cols (when n >= 257) so each matmul covers the next
    LDWEIGHTS (107 ns; walrus runs with ldw-opt off, so every matmul
    reloads its stationary operand)."""
    if n == 0:
        return []
    k = -(-n // cap)
    base, rem = divmod(n, k)
    out = []
    o = o0
    for j in range(k):
        m = base + (1 if j < rem else 0)
        out.append((o, m))
        o += m
    return out


def build_phase2(C, CA, hw_loop=0):
    _patch_tile_drain()
    nc = bass.Bass("TRN2", num_devices=NCORES, debug=False)
    CB = C - CA
    xgT = nc.dram_tensor("xgT", [ND, 128, C], BF, kind="ExternalInput")
    w1A = nc.dram_tensor("w1A", [NF, 128, D], BF, kind="ExternalInput")
    w1B = nc.dram_tensor("w1B", [NF, 128, D], BF, kind="ExternalInput")
    w2A = nc.dram_tensor("w2A", [ND, 128, F], BF, kind="ExternalInput")
    w2B = nc.dram_tensor("w2B", [ND, 128, F], BF, kind="ExternalInput")
    b1r = nc.dram_tensor("b1r", [128, 2 * NF], FP, kind="ExternalInput")
    b2r = nc.dram_tensor("b2r", [128, 2 * ND], FP, kind="ExternalInput")
    gateb = nc.dram_tensor("gateb", [128, C], FP, kind="ExternalInput")
    ygT = nc.dram_tensor("ygT", [ND, 128, C], FP, kind="ExternalOutput")

    chA = _chunks(0, CA)
    chB = _chunks(CA, CB)
    segs = [(0, chA), (1, chB)] if CB > 0 else [(0, chA)]

    with tile.TileContext(nc) as tc, ExitStack() as ctx:
        cpool = ctx.enter_context(tc.tile_pool(name="const", bufs=1))
        hpool = ctx.enter_context(tc.tile_pool(name="h", bufs=1))
        w1pool = ctx.enter_context(tc.tile_pool(name="w1", bufs=4))
        w2pool = ctx.enter_context(tc.tile_pool(name="w2", bufs=2))
        ypool = ctx.enter_context(tc.tile_pool(name="y", bufs=6))
        pspool = ctx.enter_context(tc.tile_pool(name="psum", bufs=8, space="PSUM"))

        loop_ctx = tc.For_i(0, hw_loop, 1) if hw_loop else None
        if loop_ctx is not None:
            loop_ctx.__enter__()

        xg_t = []
        for d in range(ND):
            t = cpool.tile([128, C], BF, name=f"xg{d}", tag=f"xg{d}")
            nc.scalar.dma_start(t[:], xgT.ap()[d])
            xg_t.append(t)
        gate_t = cpool.tile([128, C], FP, name="gate", tag="gate")
        nc.scalar.dma_start(gate_t[:], gateb.ap()[:])
        b1_t = cpool.tile([128, 2 * NF], FP, name="b1", tag="b1")
        nc.scalar.dma_start(b1_t[:], b1r.ap()[:])
        b2_t = cpool.tile([128, 2 * ND], FP, name="b2", tag="b2")
        nc.scalar.dma_start(b2_t[:], b2r.ap()[:])

        h_t = [
            hpool.tile([128, C], BF, name=f"h{fi}", tag=f"h{fi}")
            for fi in range(NF)
        ]

        # L1: h = gelu(x @ w1 + b1), one f-tile (128 cols of F) at a time
        w1_dram = [w1A, w1B]
        for fi in range(NF):
            wts = {}
            for s, ch in segs:
                wt = w1pool.tile([128, D], BF, name=f"w1s{s}", tag=f"w1s{s}")
                nc.sync.dma_start(wt[:], w1_dram[s].ap()[fi])
                wts[s] = wt
            pss = {
                (s, j): pspool.tile([128, n], FP, name="ps", tag="ps")
                for s, ch in segs
                for j, (o, n) in enumerate(ch)
            }
            for d in range(ND):
                for s, ch in segs:
                    for j, (o, n) in enumerate(ch):
                        nc.tensor.matmul(
                            pss[s, j][:],
                            wts[s][:, d * 128 : (d + 1) * 128],
                            xg_t[d][:, o : o + n],
                            start=(d == 0),
                            stop=(d == ND - 1),
                        )
            for s, ch in segs:
                for j, (o, n) in enumerate(ch):
                    nc.scalar.activation(
                        h_t[fi][:, o : o + n],
                        pss[s, j][:],
                        AFT.Gelu_apprx_tanh,
                        bias=b1_t[:, s * NF + fi : s * NF + fi + 1],
                        scale=1.0,
                    )

        # L2: y = gate * (h @ w2 + b2), one d-tile (128 cols of D) at a time
        w2_dram = [w2A, w2B]
        for dd in range(ND):
            wts = {}
            for s, ch in segs:
                wt = w2pool.tile([128, F], BF, name=f"w2s{s}", tag=f"w2s{s}")
                nc.sync.dma_start(wt[:], w2_dram[s].ap()[dd])
                wts[s] = wt
            qss = {
                (s, j): pspool.tile([128, n], FP, name="qs", tag="ps")
                for s, ch in segs
                for j, (o, n) in enumerate(ch)
            }
            for fi in range(NF):
                for s, ch in segs:
                    for j, (o, n) in enumerate(ch):
                        nc.tensor.matmul(
                            qss[s, j][:],
                            wts[s][:, fi * 128 : (fi + 1) * 128],
                            h_t[fi][:, o : o + n],
                            start=(fi == 0),
                            stop=(fi == NF - 1),
                        )
            for s, ch in segs:
                for j, (o, n) in enumerate(ch):
                    yt = ypool.tile([128, n], FP, name="yt", tag="yt")
                    nc.vector.tensor_scalar_add(
                        yt[:], qss[s, j][:], b2_t[:, s * ND + dd : s * ND + dd + 1]
                    )
                    nc.vector.tensor_mul(yt[:], yt[:], gate_t[:, o : o + n])
                    nc.sync.dma_start(ygT.ap()[dd][:, o : o + n], yt[:])

        if loop_ctx is not None:
            loop_ctx.__exit__(None, None, None)
    return nc


def phase2_assign(flat_idx):
    """Pack tokens into 8 cores x 2 segments. Returns (C, CA, slots) where
    slots[core] = (expert_A, ids_A, expert_B, ids_B)."""
    counts = np.bincount(flat_idx, minlength=E)
    C, CA, alloc = plan_two_segments(counts)
    CB = C - CA
    a_slots, b_slots = [], []
    for e in range(E):
        ids_e = np.nonzero(flat_idx == e)[0]
        a_e, b_e = alloc[e]
        pos = 0
        for _ in range(a_e):
            take = min(CA, len(ids_e) - pos)
            a_slots.append((e, ids_e[pos : pos + take]))
            pos += take
        for _ in range(b_e):
            take = min(CB, len(ids_e) - pos)
            b_slots.append((e, ids_e[pos : pos + take]))
            pos += take
        assert pos == len(ids_e)
    empty = np.zeros((0,), np.int64)
    while len(a_slots) < NCORES:
        a_slots.append((0, empty))
    while len(b_slots) < NCORES:
        b_slots.append((0, empty))
    slots = [
        (a_slots[k][0], a_slots[k][1], b_slots[k][0], b_slots[k][1])
        for k in range(NCORES)
    ]
    return C, CA, slots


def phase2_in_maps(x, w1, b1, w2, b2, gate, C, CA, slots):
    flat_x = x.reshape(B * S, D)
    flat_gate = gate.reshape(-1).astype(np.float32)
    used = sorted({e for ea, _, eb, _ in slots for e in (ea, eb)})
    w1p = {e: pack_w1(w1[e].astype(ml_dtypes.bfloat16)) for e in used}
    w2p = {e: pack_w2(w2[e].astype(ml_dtypes.bfloat16)) for e in used}
    b1p = {e: np.ascontiguousarray(b1[e].reshape(NF, 128).T) for e in used}
    b2p = {e: np.ascontiguousarray(b2[e].reshape(ND, 128).T) for e in used}
    in_maps = []
    for eA, idsA, eB, idsB in slots:
        xg = np.zeros((C, D), np.float32)
        gt = np.zeros((C,), np.float32)
        if len(idsA):
            xg[: len(idsA)] = flat_x[idsA]
            gt[: len(idsA)] = flat_gate[idsA]
        if len(idsB):
            xg[CA : CA + len(idsB)] = flat_x[idsB]
            gt[CA : CA + len(idsB)] = flat_gate[idsB]
        xgT = np.ascontiguousarray(
            xg.T.astype(ml_dtypes.bfloat16).reshape(ND, 128, C)
        )
        in_maps.append(
            {
                "xgT": xgT,
                "w1A": w1p[eA],
                "w1B": w1p[eB],
                "w2A": w2p[eA],
                "w2B": w2p[eB],
                "b1r": np.ascontiguousarray(np.concatenate([b1p[eA], b1p[eB]], 1)),
                "b2r": np.ascontiguousarray(np.concatenate([b2p[eA], b2p[eB]], 1)),
                "gateb": np.broadcast_to(gt[None, :], (128, C)).copy(),
            }
        )
    return in_maps


def kernel(x, router_w, w1, b1, w2, b2):
    x = np.asarray(x, np.float32)
    router_w = np.asarray(router_w, np.float32)
    w1 = np.asarray(w1, np.float32)
    b1 = np.asarray(b1, np.float32)
    w2 = np.asarray(w2, np.float32)
    b2 = np.asarray(b2, np.float32)

    idx, gate = run_phase1(x, router_w)  # [B, S] each

    flat_idx = idx.reshape(-1)
    C, CA, slots = phase2_assign(flat_idx)
    nc2 = build_phase2(C, CA)
    in_maps = phase2_in_maps(x, w1, b1, w2, b2, gate, C, CA, slots)
    res2 = _run_spmd(nc2, in_maps, list(range(NCORES)))

    y_flat = np.zeros((B * S, D), np.float32)
    for core, (eA, idsA, eB, idsB) in enumerate(slots):
        ygT = res2.results[core]["ygT"].reshape(D, C)
        if len(idsA):
            y_flat[idsA] = ygT[:, : len(idsA)].T
        if len(idsB):
            y_flat[idsB] = ygT[:, CA : CA + len(idsB)].T
    return y_flat.reshape(B, S, D)
